# revision 18
# baseline (speedup 1.0000x reference)
"""Trainium2 Bass kernel for LocalAttention: sliding-window attention gate +
per-position linear + tanh + global maxpool.

out[b,c] = tanh(max_l( sigmoid(conv1d(x, W_att) + b_att)[l] * (W_cnn @ x[b].T)[c,l] ) + b_cnn[c])

Sharding: data-parallel over batch B=64 across 8 cores (8 batches/core).

Per-core pipeline, software-skewed by SKEW batches so no engine ever waits
on the score-path DMA round trip:
  stage A(b): DMA xT (host-pretransposed bf16) -> PE 2x(2x4) matmuls with
    augmented weights [W_cnn; pad; W_att] -> ACT evacuates PSUM to SBUF bf16
    -> u rows DMA to DRAM scratch -> diagonal-strided DMA back (applies the
    sliding-window shift via DRAM strides).
  stage B(b-SKEW): ones-matmul broadcast-sum -> sigmoid -> gate multiply
    (DVE for chunk 0 / GpSimd for chunk 1, all bf16 SBUF) -> pairwise max +
    max-reduce on DVE.
"""

import functools
import sys

import ml_dtypes
import numpy as np

sys.path.insert(0, "/opt/trn_rl_repo")

import concourse.bacc as bacc
import concourse.bass as bass
import concourse.tile as tile
from concourse import mybir
from concourse.bass_utils import run_bass_kernel_spmd

B, L, E, WIN, C = 64, 1024, 512, 5, 200
NCORES = 8
BS = B // NCORES  # batches per core
P = 128
EC = E // P       # 4 contraction chunks of 128
LH = L // 2       # 512, one PSUM bank
# augmented output channels: 200 cnn + zero pad + 5 att rows at UOFF of chunk 1
UOFF = 96         # 32-aligned partition offset of W_att rows inside c-chunk 1
CAUG = P + UOFF + WIN  # 229
CCH = [(0, P), (P, UOFF + WIN)]   # (start, rows) of the two matmul chunks
CW1 = C - P                       # valid cnn rows in chunk 1 (72)
SROW = L + 4                      # DRAM scratch row length (2-col zero pad)
SKEW = 3

FP32 = mybir.dt.float32
BF16 = mybir.dt.bfloat16
AF = mybir.ActivationFunctionType
ALU = mybir.AluOpType


def _body(nc, tc, x_d, w_d, batt_d, bcnn_d, out_d):
    ones_d = nc.inline_tensor(
        np.ones((WIN, P), dtype=ml_dtypes.bfloat16), "ones5"
    ).ap()
    with (
        tc.tile_pool(name="const", bufs=1) as cpool,
        tc.tile_pool(name="xin", bufs=SKEW + 1) as xpool,
        tc.tile_pool(name="vg0", bufs=SKEW + 1) as vg0pool,
        tc.tile_pool(name="vg1", bufs=SKEW + 1) as vg1pool,
        tc.tile_pool(name="g", bufs=2) as gpool,
        tc.tile_pool(name="h", bufs=2) as hpool,
        tc.tile_pool(name="u", bufs=SKEW + 1) as upool,
        tc.tile_pool(name="s", bufs=2) as spool,
        tc.tile_pool(name="oacc", bufs=1) as opool,
        tc.tile_pool(name="dsc", bufs=1, space="DRAM") as dpool,
        tc.tile_pool(name="pv", bufs=2, space="PSUM") as pvpool,
        tc.tile_pool(name="ps", bufs=2, space="PSUM") as pspool,
    ):
        # ---- prologue: weights first, then x loads (first one split per-ec
        # so PE can start early), PE warmup matmuls to overlap the pstate
        # ramp with the initial DMAs, then the small constants. ----
        w_sb = cpool.tile([P, EC, CAUG], BF16, tag="w")
        nc.sync.dma_start(out=w_sb[:], in_=w_d.rearrange("ec p c -> p ec c"))

        xts = {}
        xts[0] = xpool.tile([P, EC, L], BF16, tag="xt", name="xt0")
        for ec in range(EC):
            nc.sync.dma_start(out=xts[0][:, ec, :], in_=x_d[0, ec])
        for b in range(1, min(SKEW, BS)):
            xts[b] = xpool.tile([P, EC, L], BF16, tag="xt", name=f"xt{b}")
            for h in range(2):
                nc.sync.dma_start(
                    out=xts[b][:, 2 * h : 2 * h + 2, :],
                    in_=x_d[b, 2 * h : 2 * h + 2].rearrange("ec p l -> p ec l"),
                )

        wpv = pvpool.tile([P, 2, LH], FP32, tag="pv", name="wpv")
        wflat = w_sb[:].rearrange("p a b -> p (a b)")
        for _ in range(6):
            nc.tensor.matmul(
                wpv[:, 0, :], lhsT=w_sb[:, 0, 0:P], rhs=wflat[:, 0:LH],
                start=True, stop=True,
            )

        ones_sb = cpool.tile([WIN, P], BF16, tag="ones")
        nc.sync.dma_start(out=ones_sb[:], in_=ones_d)
        batt_sb = cpool.tile([P, 1], FP32, tag="batt")
        nc.sync.dma_start(out=batt_sb[:], in_=batt_d)
        bcnn_sb = []
        for ci, (c0, cw) in enumerate([(0, P), (P, CW1)]):
            t = cpool.tile([cw, 1], FP32, tag=f"bcnn{ci}")
            nc.sync.dma_start(out=t[:], in_=bcnn_d[c0 : c0 + cw, :])
            bcnn_sb.append(t)

        # DRAM scratch for the sliding-window shift; zero the edge columns
        # once (per-batch writes only touch cols [2, L+2)).
        sall = dpool.tile([BS, WIN, SROW], BF16, tag="sall")
        zed = cpool.tile([WIN, 2 * BS], BF16, tag="zed")
        nc.gpsimd.memset(zed[:], 0.0)
        sbase = sall[:]
        for edge_off in (0, L + 2):
            nc.sync.dma_start(
                out=bass.AP(
                    sbase.tensor,
                    sbase.offset + edge_off,
                    [[SROW, WIN], [WIN * SROW, BS], [1, 2]],
                ),
                in_=zed[:].rearrange("p (b c) -> p b c", c=2),
            )

        oacc0 = opool.tile([P, BS], FP32, tag="oacc0")
        oacc1 = opool.tile([CW1, BS], FP32, tag="oacc1")
        vg = {}
        uali = {}

        def stage_a(b):
            xT = xts.pop(b)
            vg0 = vg0pool.tile([P, L], BF16, tag="vg0", name=f"vg0_{b}")
            vg1 = vg1pool.tile([P, L], BF16, tag="vg1", name=f"vg1_{b}")
            for ci, (c0, cw) in enumerate(CCH):
                pv = pvpool.tile([P, 2, LH], FP32, tag="pv", name=f"pv{b}_{ci}")
                for lt in range(2):
                    for ec in range(EC):
                        nc.tensor.matmul(
                            pv[:cw, lt, :],
                            lhsT=w_sb[:, ec, c0 : c0 + cw],
                            rhs=xT[:, ec, lt * LH : (lt + 1) * LH],
                            start=(ec == 0),
                            stop=(ec == EC - 1),
                        )
                t = (vg0, vg1)[ci]
                nc.scalar.copy(
                    out=t[:cw, :], in_=pv[:cw, :, :].rearrange("c a b -> c (a b)")
                )
            vg[b] = (vg0, vg1)
            # next x load, split in halves so the tiny score DMAs never
            # queue behind a full 1MB transfer on the DMA engines
            if b + SKEW < BS:
                xts[b + SKEW] = xpool.tile([P, EC, L], BF16, tag="xt", name=f"xt{b + SKEW}")
                for h in range(2):
                    nc.sync.dma_start(
                        out=xts[b + SKEW][:, 2 * h : 2 * h + 2, :],
                        in_=x_d[b + SKEW, 2 * h : 2 * h + 2].rearrange(
                            "ec p l -> p ec l"
                        ),
                    )

        def uin_dma(bp):
            uali[bp] = upool.tile([WIN, L], BF16, tag="uali", name=f"uali{bp}")
            nc.scalar.dma_start(
                out=uali[bp][:],
                in_=bass.AP(
                    sbase.tensor,
                    sbase.offset + bp * WIN * SROW,
                    [[SROW + 1, WIN], [1, L]],
                ),
            )

        def score_dmas(b):
            # issued from the ACT queue (cp1(b) just ran there) so the
            # uin->uout wait never blocks the x-load (SP) queue; uin(b) goes
            # out one iteration after uout(b).
            if b < BS:
                nc.scalar.dma_start(
                    out=sall[b, :, 2 : L + 2], in_=vg[b][1][UOFF : UOFF + WIN, :]
                )
            if 0 <= b - 1 < BS and (b - 1) not in uali:
                uin_dma(b - 1)
            if b == BS - 1:
                uin_dma(b)

        def stage_b(b):
            vg0, vg1 = vg.pop(b)
            ua = uali.pop(b)
            ps = pspool.tile([P, 2, LH], FP32, tag="ps")
            for lt in range(2):
                nc.tensor.matmul(
                    ps[:, lt, :],
                    lhsT=ones_sb[:],
                    rhs=ua[:, lt * LH : (lt + 1) * LH],
                    start=True,
                    stop=True,
                )
            ssb = spool.tile([P, L], BF16, tag="ssb")
            nc.scalar.activation(
                out=ssb[:],
                in_=ps[:, :, :].rearrange("c a b -> c (a b)"),
                func=AF.Sigmoid,
                bias=batt_sb[:],
            )
            g0 = gpool.tile([P, L], BF16, tag="g0")
            nc.vector.tensor_tensor(out=g0[:], in0=vg0[:], in1=ssb[:], op=ALU.mult)
            g1 = gpool.tile([CW1, L], BF16, tag="g1")
            nc.gpsimd.tensor_tensor(
                out=g1[:], in0=vg1[:CW1, :], in1=ssb[:CW1, :], op=ALU.mult
            )
            h0 = hpool.tile([P, LH], BF16, tag="h0")
            nc.vector.tensor_tensor(
                out=h0[:], in0=g0[:, 0:LH], in1=g0[:, LH:L], op=ALU.max
            )
            nc.vector.reduce_max(oacc0[:, b : b + 1], h0[:], axis=mybir.AxisListType.X)
            h1 = hpool.tile([CW1, LH], BF16, tag="h1")
            nc.vector.tensor_tensor(
                out=h1[:], in0=g1[:, 0:LH], in1=g1[:, LH:L], op=ALU.max
            )
            nc.vector.reduce_max(oacc1[:, b : b + 1], h1[:], axis=mybir.AxisListType.X)

        for b in range(BS):
            stage_a(b)
            score_dmas(b)
            if b >= SKEW:
                stage_b(b - SKEW)
        score_dmas(BS)
        for b in range(max(0, BS - SKEW), BS):
            stage_b(b)

        # ---- tanh(max + b_cnn) and store ----
        for ci, (c0, cw, acc) in enumerate([(0, P, oacc0), (P, CW1, oacc1)]):
            of = spool.tile([cw, BS], FP32, tag=f"of{ci}")
            nc.scalar.activation(
                out=of[:], in_=acc[:], func=AF.Tanh, bias=bcnn_sb[ci][:]
            )
            nc.sync.dma_start(out=out_d[c0 : c0 + cw, :], in_=of[:])


@functools.lru_cache(maxsize=1)
def _build():
    nc = bacc.Bacc(
        "TRN2",
        target_bir_lowering=False,
        debug=False,
        enable_asserts=False,
        num_devices=NCORES,
    )
    x_d = nc.dram_tensor("xT", [BS, EC, P, L], BF16, kind="ExternalInput").ap()
    w_d = nc.dram_tensor("waugT", [EC, P, CAUG], BF16, kind="ExternalInput").ap()
    batt_d = nc.dram_tensor("b_att_b", [P, 1], FP32, kind="ExternalInput").ap()
    bcnn_d = nc.dram_tensor("b_cnn_c", [C, 1], FP32, kind="ExternalInput").ap()
    out_d = nc.dram_tensor("out", [C, BS], FP32, kind="ExternalOutput").ap()
    with tile.TileContext(nc) as tc:
        _body(nc, tc, x_d, w_d, batt_d, bcnn_d, out_d)
    nc.compile()
    return nc


def _prep_in_maps(x, W_att, b_att, W_cnn, b_cnn):
    pad = np.zeros((CAUG - C - WIN, E), dtype=np.float32)
    waug = np.concatenate([W_cnn, pad, W_att], axis=0)     # [229, 512]
    waugT = np.ascontiguousarray(waug.T)                   # [512, 229]
    waugT = waugT.reshape(EC, P, CAUG).astype(ml_dtypes.bfloat16)
    batt = np.full((P, 1), np.float32(b_att[0]), dtype=np.float32)
    bcnn = np.asarray(b_cnn, dtype=np.float32).reshape(C, 1)
    # host-side cast + transpose: [B, L, E] -> bf16 [B, EC, 128, L]
    xb = np.asarray(x, dtype=np.float32).astype(ml_dtypes.bfloat16)
    xT = np.ascontiguousarray(xb.transpose(0, 2, 1)).reshape(B, EC, P, L)
    in_maps = []
    for c in range(NCORES):
        in_maps.append(
            {
                "xT": xT[c * BS : (c + 1) * BS],
                "waugT": waugT,
                "b_att_b": batt,
                "b_cnn_c": bcnn,
            }
        )
    return in_maps


def run(x, W_att, b_att, W_cnn, b_cnn, trace=False):
    nc = _build()
    in_maps = _prep_in_maps(x, W_att, b_att, W_cnn, b_cnn)
    res = run_bass_kernel_spmd(nc, in_maps, core_ids=list(range(NCORES)), trace=trace)
    outs = [r["out"] for r in res.results]  # each [C, BS]
    out = np.concatenate([o.T for o in outs], axis=0)  # [B, C]
    return out[:, :, None, None].astype(np.float32), res


def kernel(x, W_att, b_att, W_cnn, b_cnn):
    out, _ = run(x, W_att, b_att, W_cnn, b_cnn)
    return out
